# revision 11
# baseline (speedup 1.0000x reference)
"""Histogram-equalization kernel for Trainium2 (Bass), 8-core data parallel.

Input:  images [64, 512, 512, 3] int32 (values 0..255)
Output: [64, 512, 512, 3] uint8 — per-image per-channel histogram equalization.

Per core: 8 images = 24 independent channels of 262144 px each.

Per-channel pipeline:
  A. load + deinterleave -> x16 [128, 2048] int16 (px-partitioned)
  B. one-hot slabs: slabL[p, l*F+f] = ((x&15)==l), slabH[p, h*F+f] = ((x>>4)==h)
     (32 dual-op tensor_scalar compares, bf16 out, 4x mode)
  C. joint histogram joint[h, l] = sum_px ohHi*ohLo via F accumulating
     PE matmuls (lhsT/rhs = stride-F [128, 16] column blocks of the slabs)
  D. LUT derivation on a [1, 256] row: cumsum, step, exact integer division
     (Newton-refined reciprocal + /-1 correction), step==0 identity blend
  E. apply in transposed layout: DMA-regroup slabs to ohHiT/ohLoT
     [128 = 8 chunk x 16 slot, f], V = blockdiag(T).T @ ohLoT (PE),
     prod = ohHiT*V (DVE), out8 = 16-row group-sums (PE), uint8 planar out
     (ACT copy) -> DRAM scratch
  F. per image: reload 3 planar channels, interleave RGB (strided DVE
     copies), contiguous DMA out.
"""

import sys

sys.path.insert(0, "/opt/trn_rl_repo")

import numpy as np

P = 128
H = W = 512
CH = 3
IMG_PER_CORE = 8
N_CORES = 8
F = (H * W) // P  # 2048
NPX = H * W  # 262144
FT = 16 * F  # slab width
FC = 512  # matmul chunk (psum-bank fp32)
MMB = 2  # batches of FC per V/out8 psum tile

_cache = {}


def build(n_img=IMG_PER_CORE, debug=False):
    from contextlib import ExitStack

    import concourse.bacc as bacc
    import concourse.mybir as mybir
    from concourse.tile import TileContext

    dt = mybir.dt
    Alu = mybir.AluOpType
    AX = mybir.AxisListType

    nc = bacc.Bacc("TRN2", target_bir_lowering=False, debug=False)
    imgs = nc.dram_tensor("imgs", [n_img, H * W * CH], dt.int32, kind="ExternalInput")
    out = nc.dram_tensor("out", [n_img, H * W * CH], dt.uint8, kind="ExternalOutput")
    # planar uint8 scratch, one buffer per (img, channel)
    scratch = nc.dram_tensor("scr", [n_img * CH, H * W], dt.uint8, kind="Internal")
    dbg = None
    if debug:
        dbg = nc.dram_tensor("dbg", [n_img * CH, 256], dt.float32, kind="ExternalOutput")
        dbgh = nc.dram_tensor("dbgh", [n_img * CH, 256], dt.float32, kind="ExternalOutput")
        dbgs = nc.dram_tensor("dbgs", [n_img * CH, 8], dt.float32, kind="ExternalOutput")

    with TileContext(nc) as tc, ExitStack() as ctx:
        sb = ctx.enter_context(tc.tile_pool(name="sb", bufs=1))
        sb2 = ctx.enter_context(tc.tile_pool(name="sb2", bufs=2))
        sbd = ctx.enter_context(tc.tile_pool(name="sbd", bufs=1))
        ps = ctx.enter_context(tc.tile_pool(name="ps", bufs=1, space="PSUM"))
        ps2 = ctx.enter_context(tc.tile_pool(name="ps2", bufs=2, space="PSUM"))

        # ---- persistent constants ----
        iota_i = sb.tile([1, 256], dt.int32, tag="iota_i")
        nc.gpsimd.iota(iota_i[:], pattern=[[1, 256]], base=0, channel_multiplier=0)
        iota_f = sb.tile([1, 256], dt.float32, tag="iota_f")
        nc.vector.tensor_copy(iota_f[:], iota_i[:])

        onesBD = sb.tile([P, 8], dt.bfloat16, tag="onesBD")
        nc.vector.memset(onesBD[:], 0.0)
        ones16 = sb.tile([1, 16], dt.bfloat16, tag="ones16")
        nc.vector.memset(ones16[:], 1.0)
        for r in range(8):
            nc.sync.dma_start(
                out=onesBD[16 * r : 16 * r + 16, r : r + 1], in_=ones16[:]
            )

        bd = sb.tile([P, P], dt.bfloat16, tag="bd")
        nc.vector.memset(bd[:], 0.0)

        for img in range(n_img):
            img32 = sb.tile([P, H * W * CH // P], dt.int32, tag="img32")
            nc.sync.dma_start(out=img32[:], in_=imgs[img : img + 1, :])
            img16 = img32[:].bitcast(dt.int16)

            for c in range(CH):
                # ---- A2. deinterleave channel c ----
                x16 = sb.tile([P, F], dt.int16, tag="x16")
                nc.vector.tensor_copy(x16[:], img16[:, 2 * c :: 6])

                # ---- B. one-hot slabs ----
                lo16 = sb.tile([P, F], dt.int16, tag="lo16")
                hi16 = sb.tile([P, F], dt.int16, tag="hi16")
                nc.vector.tensor_scalar(
                    out=lo16[:], in0=x16[:], scalar1=15, scalar2=None,
                    op0=Alu.bitwise_and,
                )
                nc.vector.tensor_scalar(
                    out=hi16[:], in0=x16[:], scalar1=4, scalar2=None,
                    op0=Alu.logical_shift_right,
                )
                slabL = sb.tile([P, FT], dt.bfloat16, tag="slabL")
                slabH = sb.tile([P, FT], dt.bfloat16, tag="slabH")
                for v in range(16):
                    nc.vector.tensor_scalar(
                        out=slabL[:, v * F : (v + 1) * F],
                        in0=lo16[:], scalar1=v, scalar2=None, op0=Alu.is_equal,
                    )
                    nc.vector.tensor_scalar(
                        out=slabH[:, v * F : (v + 1) * F],
                        in0=hi16[:], scalar1=v, scalar2=None, op0=Alu.is_equal,
                    )

                # ---- C. joint histogram: joint[h, l] ----
                joint = ps2.tile([16, 16], dt.float32, tag="joint")
                for f in range(F):
                    nc.tensor.matmul(
                        out=joint[:],
                        lhsT=slabH[:, f :: F],
                        rhs=slabL[:, f :: F],
                        start=(f == 0),
                        stop=(f == F - 1),
                    )

                # ---- D. LUT on [1, 256] ----
                jnt = sbd.tile([16, 16], dt.float32, tag="jnt")
                nc.vector.tensor_copy(jnt[:], joint[:])
                histo = sbd.tile([1, 256], dt.float32, tag="histo")
                nc.gpsimd.dma_start(out=histo[:], in_=jnt[:])  # b = 16h+l order

                ca = sbd.tile([1, 256], dt.float32, tag="ca")
                cb = sbd.tile([1, 256], dt.float32, tag="cb")
                src = histo
                for k in range(8):
                    s = 1 << k
                    dst = ca if (k % 2 == 0) else cb
                    nc.vector.tensor_copy(dst[:, :s], src[:, :s])
                    nc.vector.tensor_tensor(
                        out=dst[:, s:256],
                        in0=src[:, s:256],
                        in1=src[:, : 256 - s],
                        op=Alu.add,
                    )
                    src = dst
                cum = src  # == cb after 8 iterations
                t1 = ca  # scratch distinct from cum

                # m2 = max(cum * (cum < N))
                nc.vector.tensor_scalar(
                    out=t1[:], in0=cum[:], scalar1=float(NPX), scalar2=None, op0=Alu.is_lt
                )
                nc.vector.tensor_tensor(out=t1[:], in0=t1[:], in1=cum[:], op=Alu.mult)
                m2 = sbd.tile([1, 1], dt.float32, tag="m2")
                nc.vector.tensor_reduce(out=m2[:], in_=t1[:], axis=AX.X, op=Alu.max)

                # step = floor(m2/255), exact: round-cast then correct
                # (fp32->int casts round to nearest on this HW)
                stepf = sbd.tile([1, 1], dt.float32, tag="stepf")
                nc.vector.tensor_scalar(
                    out=stepf[:], in0=m2[:], scalar1=1.0 / 255.0, scalar2=None,
                    op0=Alu.mult,
                )
                stepi = sbd.tile([1, 1], dt.int32, tag="stepi")
                nc.vector.tensor_copy(stepi[:], stepf[:])
                nc.vector.tensor_copy(stepf[:], stepi[:])
                se = sbd.tile([1, 1], dt.float32, tag="se")
                nc.vector.tensor_scalar(
                    out=se[:], in0=stepf[:], scalar1=-255.0, scalar2=None, op0=Alu.mult
                )
                nc.vector.tensor_tensor(out=se[:], in0=m2[:], in1=se[:], op=Alu.add)
                scor = sbd.tile([1, 1], dt.float32, tag="scor")
                nc.vector.tensor_scalar(
                    out=scor[:], in0=se[:], scalar1=0.0, scalar2=None, op0=Alu.is_lt
                )
                nc.vector.tensor_tensor(
                    out=stepf[:], in0=stepf[:], in1=scor[:], op=Alu.subtract
                )
                nc.vector.tensor_scalar(
                    out=scor[:], in0=se[:], scalar1=255.0, scalar2=None, op0=Alu.is_ge
                )
                nc.vector.tensor_tensor(
                    out=stepf[:], in0=stepf[:], in1=scor[:], op=Alu.add
                )
                # s = max(step, 1); half = floor(s/2) = roundcast(0.5*s - 0.25)
                s_f = sbd.tile([1, 1], dt.float32, tag="s_f")
                nc.vector.tensor_scalar(
                    out=s_f[:], in0=stepf[:], scalar1=1.0, scalar2=None, op0=Alu.max
                )
                halff = sbd.tile([1, 1], dt.float32, tag="halff")
                halfi = sbd.tile([1, 1], dt.int32, tag="halfi")
                nc.vector.tensor_scalar(
                    out=halff[:], in0=s_f[:], scalar1=0.5, scalar2=-0.25,
                    op0=Alu.mult, op1=Alu.add,
                )
                nc.vector.tensor_copy(halfi[:], halff[:])
                nc.vector.tensor_copy(halff[:], halfi[:])

                # r1 = Newton(1/s)
                r0 = sbd.tile([1, 1], dt.float32, tag="r0")
                nc.vector.reciprocal(r0[:], s_f[:])
                tn = sbd.tile([1, 1], dt.float32, tag="tn")
                nc.vector.tensor_tensor(out=tn[:], in0=s_f[:], in1=r0[:], op=Alu.mult)
                # tn = 2 - tn  ==  (tn * -1) + 2
                nc.vector.tensor_scalar(
                    out=tn[:], in0=tn[:], scalar1=-1.0, scalar2=2.0,
                    op0=Alu.mult, op1=Alu.add,
                )
                r1 = sbd.tile([1, 1], dt.float32, tag="r1")
                nc.vector.tensor_tensor(out=r1[:], in0=r0[:], in1=tn[:], op=Alu.mult)

                # cs_prev
                csp = sbd.tile([1, 256], dt.float32, tag="csp")
                nc.vector.memset(csp[:, :1], 0.0)
                nc.vector.tensor_copy(csp[:, 1:256], cum[:, :255])

                # q0 = int((csp + half) * r1)
                num = sbd.tile([1, 256], dt.float32, tag="num")
                nc.vector.tensor_scalar(
                    out=num[:], in0=csp[:], scalar1=halff[:1, :1], scalar2=r1[:1, :1],
                    op0=Alu.add, op1=Alu.mult,
                )
                q0i = sbd.tile([1, 256], dt.int32, tag="q0i")
                nc.vector.tensor_copy(q0i[:], num[:])
                q0 = sbd.tile([1, 256], dt.float32, tag="q0")
                nc.vector.tensor_copy(q0[:], q0i[:])

                # e = csp + half - q0*s; q0 += (e>=s) - (e<0); clip
                e = sbd.tile([1, 256], dt.float32, tag="e")
                nc.vector.tensor_scalar(
                    out=e[:], in0=q0[:], scalar1=s_f[:1, :1], scalar2=None, op0=Alu.mult
                )
                nc.vector.tensor_tensor(out=e[:], in0=csp[:], in1=e[:], op=Alu.subtract)
                nc.vector.tensor_scalar(
                    out=e[:], in0=e[:], scalar1=halff[:1, :1], scalar2=None, op0=Alu.add
                )
                corr = sbd.tile([1, 256], dt.float32, tag="corr")
                nc.vector.tensor_scalar(
                    out=corr[:], in0=e[:], scalar1=s_f[:1, :1], scalar2=None, op0=Alu.is_ge
                )
                nc.vector.tensor_tensor(out=q0[:], in0=q0[:], in1=corr[:], op=Alu.add)
                nc.vector.tensor_scalar(
                    out=corr[:], in0=e[:], scalar1=0.0, scalar2=None, op0=Alu.is_lt
                )
                nc.vector.tensor_tensor(out=q0[:], in0=q0[:], in1=corr[:], op=Alu.subtract)
                nc.vector.tensor_scalar(
                    out=q0[:], in0=q0[:], scalar1=0.0, scalar2=255.0,
                    op0=Alu.max, op1=Alu.min,
                )

                # step==0 -> identity
                m0 = sbd.tile([1, 1], dt.float32, tag="m0")
                nc.vector.tensor_scalar(
                    out=m0[:], in0=stepf[:], scalar1=0.0, scalar2=None, op0=Alu.is_equal
                )
                lut = sbd.tile([1, 256], dt.float32, tag="lut")
                nc.vector.tensor_tensor(out=lut[:], in0=iota_f[:], in1=q0[:], op=Alu.subtract)
                nc.vector.tensor_scalar(
                    out=lut[:], in0=lut[:], scalar1=m0[:1, :1], scalar2=None, op0=Alu.mult
                )
                nc.vector.tensor_tensor(out=lut[:], in0=lut[:], in1=q0[:], op=Alu.add)

                if debug:
                    nc.sync.dma_start(
                        out=dbg[img * CH + c : img * CH + c + 1, :], in_=lut[:]
                    )
                    nc.sync.dma_start(
                        out=dbgh[img * CH + c : img * CH + c + 1, :], in_=histo[:]
                    )
                    nc.sync.dma_start(
                        out=dbgs[img * CH + c : img * CH + c + 1, 0:1], in_=m2[:]
                    )
                    nc.sync.dma_start(
                        out=dbgs[img * CH + c : img * CH + c + 1, 1:2], in_=stepf[:]
                    )
                    nc.sync.dma_start(
                        out=dbgs[img * CH + c : img * CH + c + 1, 2:3], in_=s_f[:]
                    )
                    nc.sync.dma_start(
                        out=dbgs[img * CH + c : img * CH + c + 1, 3:4], in_=halff[:]
                    )
                    nc.sync.dma_start(
                        out=dbgs[img * CH + c : img * CH + c + 1, 4:5], in_=r1[:]
                    )

                # U[h, l] = lut[16h + l] (natural b-major), bf16; diag blocks
                Tmat = sbd.tile([16, 16], dt.bfloat16, tag="Tmat")
                nc.gpsimd.dma_start(
                    out=Tmat[:], in_=lut[:].rearrange("o (h l) -> o h l", l=16)
                )
                for r in range(8):
                    nc.gpsimd.dma_start(
                        out=bd[16 * r : 16 * r + 16, 16 * r : 16 * r + 16], in_=Tmat[:]
                    )

                # ---- E. apply ----
                sci = img * CH + c
                for C in range(16):
                    ohLoT = sb2.tile([P, F], dt.bfloat16, tag="ohLoT")
                    ohHiT = sb2.tile([P, F], dt.bfloat16, tag="ohHiT")
                    nc.sync.dma_start(
                        out=ohLoT[:],
                        in_=slabL[C::16, :].rearrange("r (l f) -> r l f", l=16),
                    )
                    nc.sync.dma_start(
                        out=ohHiT[:],
                        in_=slabH[C::16, :].rearrange("r (l f) -> r l f", l=16),
                    )
                    for b0 in range(0, F, FC * MMB):
                        V = ps.tile([P, FC * MMB], dt.float32, tag="V")
                        prod = sb2.tile([P, FC * MMB], dt.bfloat16, tag="prod")
                        out8 = ps.tile([8, FC * MMB], dt.float32, tag="out8")
                        for bi in range(MMB):
                            sl = slice(b0 + bi * FC, b0 + (bi + 1) * FC)
                            vsl = slice(bi * FC, (bi + 1) * FC)
                            nc.tensor.matmul(
                                out=V[:, vsl], lhsT=bd[:], rhs=ohHiT[:, sl],
                                start=True, stop=True,
                            )
                        for bi in range(MMB):
                            sl = slice(b0 + bi * FC, b0 + (bi + 1) * FC)
                            vsl = slice(bi * FC, (bi + 1) * FC)
                            nc.vector.tensor_tensor(
                                out=prod[:, vsl], in0=V[:, vsl], in1=ohLoT[:, sl],
                                op=Alu.mult,
                            )
                        for bi in range(MMB):
                            vsl = slice(bi * FC, (bi + 1) * FC)
                            nc.tensor.matmul(
                                out=out8[:, vsl], lhsT=onesBD[:], rhs=prod[:, vsl],
                                start=True, stop=True,
                            )
                        orgb = sb2.tile([8, FC * MMB], dt.uint8, tag="orgb")
                        nc.scalar.copy(out=orgb[:], in_=out8[:])
                        # planar scratch: rows r at byte offset r*FT + C*F + b0
                        off = C * F + b0
                        nc.sync.dma_start(
                            out=scratch[sci : sci + 1, :]
                            .rearrange("o (r x) -> o r x", x=FT)[
                                :, :, off : off + FC * MMB
                            ],
                            in_=orgb[:],
                        )

            # ---- F. interleave RGB for this image ----
            org = sb.tile([P, CH * F], dt.uint8, tag="org")
            for c in range(CH):
                pl = sb.tile([P, F], dt.uint8, tag="pl")
                nc.sync.dma_start(out=pl[:], in_=scratch[img * CH + c : img * CH + c + 1, :])
                nc.vector.tensor_copy(org[:, c :: CH], pl[:])
            nc.sync.dma_start(out=out[img : img + 1, :], in_=org[:])

    nc.compile()
    return nc


def numpy_ref_channel(img_ch):
    flat = np.asarray(img_ch).reshape(-1)
    histo = np.bincount(flat, minlength=256)
    nz = np.nonzero(histo)[0]
    last_nonzero = histo[nz[-1]] if len(nz) else 0
    step = (histo.sum() - last_nonzero) // 255
    safe_step = max(step, 1)
    lut = (np.cumsum(histo) + safe_step // 2) // safe_step
    lut = np.concatenate([[0], lut[:-1]])
    lut = np.clip(lut, 0, 255)
    if step == 0:
        return flat.reshape(img_ch.shape).astype(np.uint8)
    return lut[flat].reshape(img_ch.shape).astype(np.uint8)


def kernel(images: np.ndarray) -> np.ndarray:
    from concourse.bass_utils import run_bass_kernel_spmd

    if "nc" not in _cache:
        _cache["nc"] = build()
    nc = _cache["nc"]

    B = images.shape[0]
    flat = np.ascontiguousarray(images.reshape(B, -1).astype(np.int32))
    per = B // N_CORES
    in_maps = [{"imgs": flat[i * per : (i + 1) * per]} for i in range(N_CORES)]
    res = run_bass_kernel_spmd(nc, in_maps, core_ids=list(range(N_CORES)))
    outs = [r["out"] for r in res.results]
    return np.concatenate(outs, axis=0).reshape(B, H, W, CH).astype(np.uint8)


# revision 21
# speedup vs baseline: 1.9477x; 1.9477x over previous
"""Histogram-equalization kernel for Trainium2 (Bass), 8-core data parallel.

Input:  images [64, 512, 512, 3] int32 (values 0..255)
Output: [64, 512, 512, 3] uint8 (per-image per-channel equalization).

Per core: 8 images = 24 channels of 262144 px, as [128, 2048] int16 tiles.

This platform charges a large fixed cost per engine instruction, so the
kernel minimizes instruction count with big fused DVE ops (no PE):

  Loop 1 (per channel): deinterleave; 256-bin counts via chunked
    is_equal-vs-iota (uint8) + segmented reduce; partition fold-tree
    (64+32 TT folds, then a strided-view reduce) -> histos[ch, 256].
  Batched LUT derivation for all 24 channels on [24, 256] tiles:
    cumsum (8 shifted adds), exact step = floor(m2/255) and
    lut = floor((csprev + half)/step) via round-cast + integer residual
    correction (the fp32->int cast rounds to nearest), step==0 identity.
  Loop 2 (per channel): lut applied as out = sum_h [hi==h] * W_h,
    W_h = sum_l T[h,l]*[lo==l], chunked; all products have exactly one
    nonzero term so bf16 stays exact. Strided uint8 write interleaves RGB.
"""

import sys

sys.path.insert(0, "/opt/trn_rl_repo")

import numpy as np

P = 128
H = W = 512
CH = 3
IMG_PER_CORE = 8
N_CORES = 8
F = (H * W) // P  # 2048
NPX = H * W
FH = 128  # histogram chunk: 256*FH = 32768 fits 16-bit ISA fields
FA = 256  # apply chunk (prod tile [128, 16*FA*16] uint8 = 64KB/part)

_cache = {}


def build(n_img=IMG_PER_CORE, debug=False):
    from contextlib import ExitStack

    import concourse.bacc as bacc
    import concourse.mybir as mybir
    from concourse.tile import TileContext

    dt = mybir.dt
    Alu = mybir.AluOpType
    AX = mybir.AxisListType

    nch = n_img * CH
    nc = bacc.Bacc("TRN2", target_bir_lowering=False, debug=False)
    imgs = nc.dram_tensor("imgs", [n_img, H * W * CH], dt.int32, kind="ExternalInput")
    out = nc.dram_tensor("out", [n_img, H * W * CH], dt.uint8, kind="ExternalOutput")
    dbg = None
    if debug:
        dbg = nc.dram_tensor("dbg", [nch, 256], dt.float32, kind="ExternalOutput")

    with TileContext(nc) as tc, ExitStack() as ctx:
        sb = ctx.enter_context(tc.tile_pool(name="sb", bufs=1))
        sbd = ctx.enter_context(tc.tile_pool(name="sbd", bufs=1))

        # constants materialized on all partitions (cm=0)
        iota256 = sb.tile([P, 256], dt.int16, tag="iota256")
        nc.gpsimd.iota(iota256[:], pattern=[[1, 256]], base=0, channel_multiplier=0)
        iotaL = sb.tile([P, 16], dt.int16, tag="iotaL")
        nc.gpsimd.iota(iotaL[:], pattern=[[1, 16]], base=0, channel_multiplier=0)
        iotaf = sbd.tile([nch, 256], dt.float32, tag="iotaf")
        ioti = sbd.tile([nch, 256], dt.int32, tag="ioti")
        nc.gpsimd.iota(ioti[:], pattern=[[1, 256]], base=0, channel_multiplier=0)
        nc.vector.tensor_copy(iotaf[:], ioti[:])

        histos = sbd.tile([nch, 256], dt.float32, tag="histos")

        # ---------- Loop 1: histograms ----------
        for img in range(n_img):
            img32 = sb.tile([P, H * W * CH // P], dt.int32, tag="img32")
            nc.sync.dma_start(out=img32[:], in_=imgs[img : img + 1, :])
            img16 = img32[:].bitcast(dt.int16)
            for c in range(CH):
                ch = img * CH + c
                x16 = sb.tile([P, F], dt.int16, tag="x16")
                nc.vector.tensor_copy(x16[:], img16[:, 2 * c :: 6])

                part = sb.tile([P, 256], dt.uint16, tag="part")
                for k in range(F // FH):
                    eq = sb.tile([P, 256 * FH], dt.uint8, tag="big")
                    # eq[p, b*FH + f] = (x16[p, k*FH + f] == b)
                    nc.vector.tensor_tensor(
                        out=eq[:],
                        in0=x16[:, k * FH : (k + 1) * FH]
                        .unsqueeze(1)
                        .to_broadcast([P, 256, FH]),
                        in1=iota256[:].unsqueeze(2).to_broadcast([P, 256, FH]),
                        op=Alu.is_equal,
                    )
                    pk = sb.tile([P, 256], dt.uint16, tag="pk")
                    with nc.allow_low_precision(
                        reason="integer counts <= 256 fit uint16 exactly"
                    ):
                        nc.vector.tensor_reduce(
                            out=pk[:],
                            in_=eq[:].rearrange("p (b f) -> p b f", f=FH),
                            axis=AX.X,
                            op=Alu.add,
                        )
                    if k == 0:
                        nc.vector.tensor_copy(part[:], pk[:])
                    else:
                        nc.vector.tensor_tensor(
                            out=part[:], in0=part[:], in1=pk[:], op=Alu.add
                        )
                # gather all 128 rows into one row, reduce with strided view
                row128 = sb.tile([1, P * 256], dt.uint16, tag="row128")
                nc.sync.dma_start(out=row128[:], in_=part[:])
                # row128[0, p*256 + b]; reduce over p via [1, 256(b), 128(p)]
                hrow = sb.tile([1, 256], dt.float32, tag="hrow")
                nc.vector.tensor_reduce(
                    out=hrow[:],
                    in_=row128[:].rearrange("o (pp b) -> o b pp", b=256),
                    axis=AX.X,
                    op=Alu.add,
                )
                nc.sync.dma_start(out=histos[ch : ch + 1, :], in_=hrow[:])

        # ---------- Batched LUT derivation on [nch, 256] ----------
        NC2 = nch
        ca = sbd.tile([NC2, 256], dt.float32, tag="ca")
        cb = sbd.tile([NC2, 256], dt.float32, tag="cb")
        src = histos
        for k in range(8):
            s = 1 << k
            dst = ca if (k % 2 == 0) else cb
            nc.vector.tensor_copy(dst[:, :s], src[:, :s])
            nc.vector.tensor_tensor(
                out=dst[:, s:256], in0=src[:, s:256], in1=src[:, : 256 - s],
                op=Alu.add,
            )
            src = dst
        cum = src  # cb
        t1 = ca

        nc.vector.tensor_scalar(
            out=t1[:], in0=cum[:], scalar1=float(NPX), scalar2=None, op0=Alu.is_lt
        )
        nc.vector.tensor_tensor(out=t1[:], in0=t1[:], in1=cum[:], op=Alu.mult)
        m2 = sbd.tile([NC2, 1], dt.float32, tag="m2")
        nc.vector.tensor_reduce(out=m2[:], in_=t1[:], axis=AX.X, op=Alu.max)

        stepf = sbd.tile([NC2, 1], dt.float32, tag="stepf")
        nc.vector.tensor_scalar(
            out=stepf[:], in0=m2[:], scalar1=1.0 / 255.0, scalar2=None, op0=Alu.mult
        )
        stepi = sbd.tile([NC2, 1], dt.int32, tag="stepi")
        nc.vector.tensor_copy(stepi[:], stepf[:])
        nc.vector.tensor_copy(stepf[:], stepi[:])
        se = sbd.tile([NC2, 1], dt.float32, tag="se")
        nc.vector.tensor_scalar(
            out=se[:], in0=stepf[:], scalar1=-255.0, scalar2=None, op0=Alu.mult
        )
        nc.vector.tensor_tensor(out=se[:], in0=m2[:], in1=se[:], op=Alu.add)
        scor = sbd.tile([NC2, 1], dt.float32, tag="scor")
        nc.vector.tensor_scalar(
            out=scor[:], in0=se[:], scalar1=0.0, scalar2=None, op0=Alu.is_lt
        )
        nc.vector.tensor_tensor(
            out=stepf[:], in0=stepf[:], in1=scor[:], op=Alu.subtract
        )
        nc.vector.tensor_scalar(
            out=scor[:], in0=se[:], scalar1=255.0, scalar2=None, op0=Alu.is_ge
        )
        nc.vector.tensor_tensor(out=stepf[:], in0=stepf[:], in1=scor[:], op=Alu.add)

        s_f = sbd.tile([NC2, 1], dt.float32, tag="s_f")
        nc.vector.tensor_scalar(
            out=s_f[:], in0=stepf[:], scalar1=1.0, scalar2=None, op0=Alu.max
        )
        halff = sbd.tile([NC2, 1], dt.float32, tag="halff")
        halfi = sbd.tile([NC2, 1], dt.int32, tag="halfi")
        nc.vector.tensor_scalar(
            out=halff[:], in0=s_f[:], scalar1=0.5, scalar2=-0.25,
            op0=Alu.mult, op1=Alu.add,
        )
        nc.vector.tensor_copy(halfi[:], halff[:])
        nc.vector.tensor_copy(halff[:], halfi[:])

        r0 = sbd.tile([NC2, 1], dt.float32, tag="r0")
        nc.vector.reciprocal(r0[:], s_f[:])
        tn = sbd.tile([NC2, 1], dt.float32, tag="tn")
        nc.vector.tensor_tensor(out=tn[:], in0=s_f[:], in1=r0[:], op=Alu.mult)
        nc.vector.tensor_scalar(
            out=tn[:], in0=tn[:], scalar1=-1.0, scalar2=2.0, op0=Alu.mult, op1=Alu.add
        )
        r1 = sbd.tile([NC2, 1], dt.float32, tag="r1")
        nc.vector.tensor_tensor(out=r1[:], in0=r0[:], in1=tn[:], op=Alu.mult)

        csp = sbd.tile([NC2, 256], dt.float32, tag="csp")
        nc.vector.memset(csp[:, :1], 0.0)
        nc.vector.tensor_copy(csp[:, 1:256], cum[:, :255])

        num = sbd.tile([NC2, 256], dt.float32, tag="num")
        nc.vector.tensor_scalar(
            out=num[:], in0=csp[:], scalar1=halff[:, :1], scalar2=r1[:, :1],
            op0=Alu.add, op1=Alu.mult,
        )
        q0i = sbd.tile([NC2, 256], dt.int32, tag="q0i")
        nc.vector.tensor_copy(q0i[:], num[:])
        q0 = sbd.tile([NC2, 256], dt.float32, tag="q0")
        nc.vector.tensor_copy(q0[:], q0i[:])

        e = sbd.tile([NC2, 256], dt.float32, tag="e")
        nc.vector.tensor_scalar(
            out=e[:], in0=q0[:], scalar1=s_f[:, :1], scalar2=None, op0=Alu.mult
        )
        nc.vector.tensor_tensor(out=e[:], in0=csp[:], in1=e[:], op=Alu.subtract)
        nc.vector.tensor_scalar(
            out=e[:], in0=e[:], scalar1=halff[:, :1], scalar2=None, op0=Alu.add
        )
        corr = sbd.tile([NC2, 256], dt.float32, tag="corr")
        nc.vector.tensor_scalar(
            out=corr[:], in0=e[:], scalar1=s_f[:, :1], scalar2=None, op0=Alu.is_ge
        )
        nc.vector.tensor_tensor(out=q0[:], in0=q0[:], in1=corr[:], op=Alu.add)
        nc.vector.tensor_scalar(
            out=corr[:], in0=e[:], scalar1=0.0, scalar2=None, op0=Alu.is_lt
        )
        nc.vector.tensor_tensor(out=q0[:], in0=q0[:], in1=corr[:], op=Alu.subtract)
        nc.vector.tensor_scalar(
            out=q0[:], in0=q0[:], scalar1=0.0, scalar2=255.0, op0=Alu.max, op1=Alu.min
        )

        m0 = sbd.tile([NC2, 1], dt.float32, tag="m0")
        nc.vector.tensor_scalar(
            out=m0[:], in0=stepf[:], scalar1=0.0, scalar2=None, op0=Alu.is_equal
        )
        lut = sbd.tile([NC2, 256], dt.float32, tag="lut")
        nc.vector.tensor_tensor(out=lut[:], in0=iotaf[:], in1=q0[:], op=Alu.subtract)
        nc.vector.tensor_scalar(
            out=lut[:], in0=lut[:], scalar1=m0[:, :1], scalar2=None, op0=Alu.mult
        )
        nc.vector.tensor_tensor(out=lut[:], in0=lut[:], in1=q0[:], op=Alu.add)
        lutb = sbd.tile([NC2, 256], dt.uint8, tag="lutb")
        nc.vector.tensor_copy(lutb[:], lut[:])
        if debug:
            nc.sync.dma_start(out=dbg[:, :], in_=lut[:])

        # ---------- Loop 2: apply ----------
        for img in range(n_img):
            img32b = sb.tile([P, H * W * CH // P], dt.int32, tag="img32")
            nc.sync.dma_start(out=img32b[:], in_=imgs[img : img + 1, :])
            img16b = img32b[:].bitcast(dt.int16)
            org = sb.tile([P, CH * F], dt.uint8, tag="org")
            for c in range(CH):
                ch = img * CH + c
                x16 = sb.tile([P, F], dt.int16, tag="x16")
                nc.vector.tensor_copy(x16[:], img16b[:, 2 * c :: 6])
                lo16 = sb.tile([P, F], dt.int16, tag="lo16")
                hi16 = sb.tile([P, F], dt.int16, tag="hi16")
                nc.vector.tensor_scalar(
                    out=lo16[:], in0=x16[:], scalar1=15, scalar2=None,
                    op0=Alu.bitwise_and,
                )
                nc.vector.tensor_scalar(
                    out=hi16[:], in0=x16[:], scalar1=4, scalar2=None,
                    op0=Alu.logical_shift_right,
                )
                # replicate this channel's lut row to all partitions, bf16
                T128 = sb.tile([P, 256], dt.uint8, tag="T128")
                nc.sync.dma_start(
                    out=T128[:],
                    in_=lutb[ch : ch + 1, :].unsqueeze(1).to_broadcast([1, P, 256]),
                )
                outb = sb.tile([P, F], dt.uint8, tag="outb")
                for k in range(F // FA):
                    sl = slice(k * FA, (k + 1) * FA)
                    # slabL chunk [P, 16l * FA] (l-major)
                    slabLc = sb.tile([P, 16 * FA], dt.uint8, tag="slabLc")
                    nc.vector.tensor_tensor(
                        out=slabLc[:],
                        in0=lo16[:, sl].unsqueeze(1).to_broadcast([P, 16, FA]),
                        in1=iotaL[:].unsqueeze(2).to_broadcast([P, 16, FA]),
                        op=Alu.is_equal,
                    )
                    slabHc = sb.tile([P, 16 * FA], dt.uint8, tag="slabHc")
                    nc.vector.tensor_tensor(
                        out=slabHc[:],
                        in0=hi16[:, sl].unsqueeze(1).to_broadcast([P, 16, FA]),
                        in1=iotaL[:].unsqueeze(2).to_broadcast([P, 16, FA]),
                        op=Alu.is_equal,
                    )
                    # prod[p, (h, f, l)] = slabLc[p, l*FA + f] * T128[p, 16h + l]
                    prod = sb.tile([P, 16 * FA * 16], dt.uint8, tag="big")
                    half = 8 * FA * 16
                    for hh in range(2):
                        nc.vector.tensor_tensor(
                            out=prod[:, hh * half : (hh + 1) * half],
                            in0=slabLc[:]
                            .rearrange("p (l f) -> p f l", l=16)
                            .unsqueeze(1)
                            .to_broadcast([P, 8, FA, 16]),
                            in1=T128[:, hh * 128 : (hh + 1) * 128]
                            .rearrange("p (h l) -> p h l", l=16)
                            .unsqueeze(2)
                            .to_broadcast([P, 8, FA, 16]),
                            op=Alu.mult,
                        )
                    # W[p, (h, f)] = sum_l prod
                    Wc = sb.tile([P, 16 * FA], dt.uint8, tag="Wc")
                    with nc.allow_low_precision(
                        reason="sums have exactly one nonzero bf16 term"
                    ):
                        nc.vector.tensor_reduce(
                        out=Wc[:],
                            in_=prod[:].rearrange(
                                "p (h f l) -> p (h f) l", l=16, f=FA
                            ),
                            axis=AX.X,
                            op=Alu.add,
                        )
                    # prod2[p, (f, h)] = slabHc * Wc (both (h, f) viewed as (f, h))
                    prod2 = sb.tile([P, FA * 16], dt.uint8, tag="prod2")
                    nc.vector.tensor_tensor(
                        out=prod2[:],
                        in0=slabHc[:].rearrange("p (h f) -> p f h", h=16),
                        in1=Wc[:].rearrange("p (h f) -> p f h", h=16),
                        op=Alu.mult,
                    )
                    with nc.allow_low_precision(
                        reason="sums have exactly one nonzero bf16 term"
                    ):
                        nc.vector.tensor_reduce(
                            out=outb[:, sl],
                            in_=prod2[:].rearrange("p (f h) -> p f h", h=16),
                            axis=AX.X,
                            op=Alu.add,
                        )
                # interleave into RGB layout (strided uint8 write)
                nc.vector.tensor_copy(org[:, c :: CH], outb[:])
            nc.sync.dma_start(out=out[img : img + 1, :], in_=org[:])

    nc.compile()
    return nc


def numpy_ref_channel(img_ch):
    flat = np.asarray(img_ch).reshape(-1)
    histo = np.bincount(flat, minlength=256)
    nz = np.nonzero(histo)[0]
    last_nonzero = histo[nz[-1]] if len(nz) else 0
    step = (histo.sum() - last_nonzero) // 255
    safe_step = max(step, 1)
    lut = (np.cumsum(histo) + safe_step // 2) // safe_step
    lut = np.concatenate([[0], lut[:-1]])
    lut = np.clip(lut, 0, 255)
    if step == 0:
        return flat.reshape(img_ch.shape).astype(np.uint8)
    return lut[flat].reshape(img_ch.shape).astype(np.uint8)


def kernel(images: np.ndarray) -> np.ndarray:
    from concourse.bass_utils import run_bass_kernel_spmd

    if "nc" not in _cache:
        _cache["nc"] = build()
    nc = _cache["nc"]

    B = images.shape[0]
    flat = np.ascontiguousarray(images.reshape(B, -1).astype(np.int32))
    per = B // N_CORES
    in_maps = [{"imgs": flat[i * per : (i + 1) * per]} for i in range(N_CORES)]
    res = run_bass_kernel_spmd(nc, in_maps, core_ids=list(range(N_CORES)))
    outs = [r["out"] for r in res.results]
    return np.concatenate(outs, axis=0).reshape(B, H, W, CH).astype(np.uint8)
